# revision 2
# baseline (speedup 1.0000x reference)
"""Trainium2 Bass kernel for nn_HWC_SpatialAttention.

Reference computation (per (b,s) slice, hw = H*W = 1024, c = 256):
    img  = img_feat[b,s]   as [hw, c1]   (DRAM holds the transpose [c1, hw])
    dep  = depth_feat[b,s] as [hw, c2]
    q = img @ Wq + bq ; k = dep @ Wk + bk ; v = dep @ Wv + bv
    attn = softmax(q @ k^T / 16)
    out  = attn @ v + img            -> returned as [c, hw]

Sharding: 32 (b,s) slices, 4 per NeuronCore, weights replicated. No
collectives. All matmuls run in float32r (TF32-class precision).

Per-slice dataflow (all layouts chosen so no transposes are needed):
    qT[c,hw]  = Wq^T-contract img:  lhsT=Wq[c1,c] tiles, rhs=X=imgT[c1,hw]
    kT[c,hw]  likewise from depT
    v[hw,c]   = lhsT=depT[c2,hw] tiles (data stationary), rhs=Wv[c2,c]
    scoresT[k,q] = lhsT=kT tiles, rhs=qT; exp fused into the PSUM
        eviction on the scalar engine (scale=1/16), bias bq/bk fused too.
    denom[1,q] = ones[k,1]^T-contract expT  (accumulated over k tiles)
    bcast[128,q] = ones[1,128]^T @ denom  (K=1 matmul), reciprocal on DVE
    outT[c,q] = lhsT=v[k,c] tiles, rhs=expT[k,q]  (accumulate over k)
    final = outT * rden + (bv + imgT)   (two DVE ops), DMA out.

bv is folded into the residual because sum_k attn_norm = 1.
"""

import numpy as np

import concourse.bass as bass
import concourse.tile as tile
from concourse import mybir
from concourse.bass_utils import run_bass_kernel_spmd

DT = mybir.dt

N_CORES = 8
B, S, C, HW = 4, 8, 256, 1024
SLICES = B * S
SPC = SLICES // N_CORES      # slices per core
CT = C // 128                # c tiles (2)
KT = HW // 128               # hw tiles (8)
NH = HW // 512               # 512-wide q chunks (2)

# ---------------------------------------------------------------------------
# walrus's CoreV3 codegen rejects instructions carrying more than one
# sync-wait command (and its fp32/fp32r matmul lowering adds one of its own
# to the generated LDWEIGHTS). Split excess waits onto same-engine nops
# inserted immediately before the over-limit instruction.
_WAIT_LIMIT = 1


def _split_excess_waits(nc):
    ctr = 0
    for f in nc.m.functions:
        for blk in f.blocks:
            new = []
            changed = False
            for inst in blk.instructions:
                si = getattr(inst, "sync_info", None)
                waits = list(si.on_wait) if si and si.on_wait else []
                if len(waits) > _WAIT_LIMIT and inst.engine != mybir.EngineType.Unassigned:
                    extra, keep = waits[:-_WAIT_LIMIT], waits[-_WAIT_LIMIT:]
                    for i in range(len(extra)):
                        ctr += 1
                        nop = mybir.InstNoOp(
                            name=f"I-waitsplit-{ctr}",
                            engine=inst.engine,
                            ins=[], outs=[],
                            sync_info=mybir.SyncInfo(on_wait=[extra[i]], on_update=[]),
                            bass_nofuse=True,
                        )
                        nc.register_instruction(nop)
                        new.append(nop)
                    inst.sync_info = mybir.SyncInfo(on_wait=keep, on_update=si.on_update)
                    changed = True
                new.append(inst)
            if changed:
                blk.instructions = new


class _TC(tile.TileContext):
    def _drain_and_barrier(self, tick_clock, wait_clock):
        nc = self.nc
        drain_inst = nc.sync.drain()
        wait_clock.add_sem_waits(
            drain_inst.ins, tile.ScopedClock({None: tick_clock.global_clock})
        )
        nc.all_engine_barrier()
        assert self.sems is not None
        popped = nc._tile_sem_poison_stack.pop()
        assert popped is self._sem_poison
        nc.clear_and_free_semaphores(list(self.sems.allocated().values()))
        nc.all_engine_barrier()
        _split_excess_waits(nc)


# ---------------------------------------------------------------------------

def _build_program():
    nc = bass.Bass("TRN2", target_bir_lowering=False, debug=False, num_devices=1)

    img_ap = nc.dram_tensor("img", [SPC, C, HW], DT.float32r, kind="ExternalInput").ap()
    dep_ap = nc.dram_tensor("dep", [SPC, C, HW], DT.float32r, kind="ExternalInput").ap()
    wq_ap = nc.dram_tensor("wq", [C, C], DT.float32r, kind="ExternalInput").ap()
    wk_ap = nc.dram_tensor("wk", [C, C], DT.float32r, kind="ExternalInput").ap()
    wv_ap = nc.dram_tensor("wv", [C, C], DT.float32r, kind="ExternalInput").ap()
    bq_ap = nc.dram_tensor("bq", [CT, 128], DT.float32, kind="ExternalInput").ap()
    bk_ap = nc.dram_tensor("bk", [CT, 128], DT.float32, kind="ExternalInput").ap()
    bv_ap = nc.dram_tensor("bv", [CT, 128], DT.float32, kind="ExternalInput").ap()
    ones_kc_ap = nc.dram_tensor("ones_kc", [128, 1], DT.float32r, kind="ExternalInput").ap()
    ones_bc_ap = nc.dram_tensor("ones_bc", [1, 128], DT.float32r, kind="ExternalInput").ap()
    out_ap = nc.dram_tensor("out", [SPC, C, HW], DT.float32, kind="ExternalOutput").ap()

    Exp = mybir.ActivationFunctionType.Exp
    Ident = mybir.ActivationFunctionType.Identity
    SCALE = 1.0 / 16.0  # 1/sqrt(C)

    with _TC(nc) as tc:
        from contextlib import ExitStack
        with ExitStack() as ctx:
            const = ctx.enter_context(tc.tile_pool(name="const", bufs=1))
            io_pool = ctx.enter_context(tc.tile_pool(name="io", bufs=2))
            qk_pool = ctx.enter_context(tc.tile_pool(name="qk", bufs=2))
            v_pool = ctx.enter_context(tc.tile_pool(name="vp", bufs=2))
            exp_pool = ctx.enter_context(tc.tile_pool(name="expp", bufs=2))
            den_pool = ctx.enter_context(tc.tile_pool(name="denp", bufs=2))
            out_pool = ctx.enter_context(tc.tile_pool(name="outp", bufs=4))
            ps_proj = ctx.enter_context(tc.tile_pool(name="ps_proj", bufs=2, space="PSUM"))
            ps_sc = ctx.enter_context(tc.tile_pool(name="ps_sc", bufs=2, space="PSUM"))
            ps_av = ctx.enter_context(tc.tile_pool(name="ps_av", bufs=2, space="PSUM"))
            ps_den = ctx.enter_context(tc.tile_pool(name="ps_den", bufs=1, space="PSUM"))
            ps_bc = ctx.enter_context(tc.tile_pool(name="ps_bc", bufs=1, space="PSUM"))

            # --- constants (once) ---
            wq = const.tile([128, CT, C], DT.float32r)
            nc.sync.dma_start(wq[:], wq_ap.rearrange("(t p) m -> p t m", p=128))
            wk = const.tile([128, CT, C], DT.float32r)
            nc.sync.dma_start(wk[:], wk_ap.rearrange("(t p) m -> p t m", p=128))
            wv = const.tile([128, CT, C], DT.float32r)
            nc.sync.dma_start(wv[:], wv_ap.rearrange("(t p) m -> p t m", p=128))
            bq = const.tile([128, CT], DT.float32)
            nc.sync.dma_start(bq[:], bq_ap.rearrange("t p -> p t"))
            bk = const.tile([128, CT], DT.float32)
            nc.sync.dma_start(bk[:], bk_ap.rearrange("t p -> p t"))
            bv = const.tile([128, CT], DT.float32)
            nc.sync.dma_start(bv[:], bv_ap.rearrange("t p -> p t"))
            ones_kc = const.tile([128, 1], DT.float32r)
            nc.sync.dma_start(ones_kc[:], ones_kc_ap[:])
            ones_bc = const.tile([1, 128], DT.float32r)
            nc.sync.dma_start(ones_bc[:], ones_bc_ap[:])

            for s in range(SPC):
                # --- load inputs, [c,hw] channel-major, c split into 2 tiles
                xs = io_pool.tile([128, CT, HW], DT.float32r, name="xs")
                nc.sync.dma_start(xs[:], img_ap[s].rearrange("(t p) n -> p t n", p=128))
                ds = io_pool.tile([128, CT, HW], DT.float32r, name="ds")
                nc.sync.dma_start(ds[:], dep_ap[s].rearrange("(t p) n -> p t n", p=128))

                # --- q/k projections -> qT/kT [c, hw] (f32r, bias fused) ---
                qT = qk_pool.tile([128, CT, HW], DT.float32r, name="qT")
                kT = qk_pool.tile([128, CT, HW], DT.float32r, name="kT")
                for dst, w, b, src in ((qT, wq, bq, xs), (kT, wk, bk, ds)):
                    for ct in range(CT):
                        for nh in range(NH):
                            pt = ps_proj.tile([128, 512], DT.float32, name="ps_proj")
                            for kt in range(CT):
                                nc.tensor.matmul(
                                    pt[:], w[:, kt, 128 * ct:128 * (ct + 1)],
                                    src[:, kt, 512 * nh:512 * (nh + 1)],
                                    start=(kt == 0), stop=(kt == CT - 1))
                            nc.scalar.activation(
                                dst[:, ct, 512 * nh:512 * (nh + 1)], pt[:],
                                Ident, bias=b[:, ct:ct + 1])

                # --- v projection -> v [hw, c] (f32r, no bias: folded at end)
                v = v_pool.tile([128, KT, C], DT.float32r, name="v")
                for mt in range(KT):
                    pt = ps_proj.tile([128, 512], DT.float32, name="ps_proj")
                    for kt in range(CT):
                        nc.tensor.matmul(
                            pt[:, :C], ds[:, kt, 128 * mt:128 * (mt + 1)],
                            wv[:, kt, :], start=(kt == 0), stop=(kt == CT - 1))
                    nc.scalar.copy(v[:, mt, :], pt[:, :C])

                # --- attention, processed per 512-wide q chunk ---
                expT = exp_pool.tile([128, KT, HW], DT.float32r, name="expT")
                rden = den_pool.tile([128, HW], DT.float32, name="rden")
                den_sb = den_pool.tile([1, HW], DT.float32r, name="den_sb")
                for nh in range(NH):
                    qs = slice(512 * nh, 512 * (nh + 1))
                    dn = ps_den.tile([1, 512], DT.float32, name="ps_den")
                    for mt in range(KT):
                        # scoresT tile [k=128 of mt, q=512 of nh]
                        pt = ps_sc.tile([128, 512], DT.float32, name="ps_sc")
                        for ct in range(CT):
                            nc.tensor.matmul(
                                pt[:], kT[:, ct, 128 * mt:128 * (mt + 1)],
                                qT[:, ct, qs], start=(ct == 0), stop=(ct == CT - 1))
                        # fused exp(score/16) eviction
                        nc.scalar.activation(expT[:, mt, qs], pt[:], Exp, scale=SCALE)
                        # denominator partial sums (accumulate over mt)
                        nc.tensor.matmul(
                            dn[:], ones_kc[:], expT[:, mt, qs],
                            start=(mt == 0), stop=(mt == KT - 1),
                            skip_group_check=True)
                    nc.scalar.copy(den_sb[:, qs], dn[:])
                    # broadcast denom across 128 partitions (K=1 matmul)
                    bc = ps_bc.tile([128, 512], DT.float32, name="ps_bc")
                    nc.tensor.matmul(bc[:], ones_bc[:], den_sb[:, qs],
                                     start=True, stop=True)
                    nc.vector.reciprocal(rden[:, qs], bc[:])

                    # --- attn @ v -> outT [c, q-chunk], then normalize+resid
                    for ct in range(CT):
                        po = ps_av.tile([128, 512], DT.float32, name="ps_av")
                        for mt in range(KT):
                            nc.tensor.matmul(
                                po[:], v[:, mt, 128 * ct:128 * (ct + 1)],
                                expT[:, mt, qs], start=(mt == 0), stop=(mt == KT - 1))
                        o = out_pool.tile([128, 512], DT.float32, name="o")
                        nc.vector.tensor_mul(o[:], po[:], rden[:, qs])
                        nc.vector.scalar_tensor_tensor(
                            o[:], o[:], bv[:, ct:ct + 1],
                            xs[:, ct, qs].bitcast(DT.float32),
                            op0=mybir.AluOpType.add, op1=mybir.AluOpType.add)
                        nc.sync.dma_start(
                            out_ap[s].rearrange("(t p) n -> p t n", p=128)[:, ct, qs],
                            o[:])
    return nc


_PROGRAM = None


def _get_program():
    global _PROGRAM
    if _PROGRAM is None:
        _PROGRAM = _build_program()
    return _PROGRAM


LAST_RESULT = None  # set by kernel(); lets a test harness read exec_time_ns


def kernel(img_feat, depth_feat, Wq, bq, Wk, bk, Wv, bv):
    global LAST_RESULT
    img = np.ascontiguousarray(img_feat, dtype=np.float32).reshape(SLICES, C, HW)
    dep = np.ascontiguousarray(depth_feat, dtype=np.float32).reshape(SLICES, C, HW)
    wq = np.ascontiguousarray(Wq, dtype=np.float32)
    wk = np.ascontiguousarray(Wk, dtype=np.float32)
    wv = np.ascontiguousarray(Wv, dtype=np.float32)
    bq2 = np.ascontiguousarray(bq, dtype=np.float32).reshape(CT, 128)
    bk2 = np.ascontiguousarray(bk, dtype=np.float32).reshape(CT, 128)
    bv2 = np.ascontiguousarray(bv, dtype=np.float32).reshape(CT, 128)
    ones_kc = np.ones((128, 1), dtype=np.float32)
    ones_bc = np.ones((1, 128), dtype=np.float32)

    nc = _get_program()
    in_maps = [
        {
            "img": img[SPC * i:SPC * (i + 1)],
            "dep": dep[SPC * i:SPC * (i + 1)],
            "wq": wq, "wk": wk, "wv": wv,
            "bq": bq2, "bk": bk2, "bv": bv2,
            "ones_kc": ones_kc, "ones_bc": ones_bc,
        }
        for i in range(N_CORES)
    ]
    res = run_bass_kernel_spmd(nc, in_maps, list(range(N_CORES)))
    LAST_RESULT = res
    out = np.concatenate([res.results[i]["out"] for i in range(N_CORES)], axis=0)
    return out.reshape(B, S, C, 32, 32).astype(img_feat.dtype)
